# revision 7
# baseline (speedup 1.0000x reference)
"""InteractionMapInit Trainium2 kernel.

out[i, j, :] = tanh( (X@Wt + bt)[i] - (Dft@Wd + bd)[j] + dnorm[i, j] )  if seg_res[i] == seg_atom[j]
             = 0                                                        otherwise

One block (DT pair) per core, SPMD over 8 cores; the host scatters the 8
dense blocks into the zeros [NR, NA, H] output (the mask is block-diagonal,
so everything off-block is zero and never touches the device).

Device program per core (Rp padded rows, Ap padded atoms):
  - tfT [H, Rp] = Wt.T @ X.T        f32r matmuls (1 cyc/row at N>=256)
  - D [128, Ap] per row tile via a rank-5 distance factorization; per-block
    dmin/dmax via DVE free-dim reduces + GPSIMD partition_all_reduce(max);
    dnorm^T = (D - dmin)/(dmax - dmin) transposed into lhsT2 rows 0..Ap-1
  - main loop per 128-row tile, 3-bank PSUM groups (1536 cols):
      mm1 (lhsT = tfT slice, rhs = I4 tiled identity)   -> + tf[i, h]
      mm2 (lhsT = [dnorm^T; ones], rhs = R2D slice)     -> + dnorm[i,j]
                                                           + bt[h] - df[j,h]
      ACT tanh PSUM -> fp16 SBUF, DMA out (rows are contiguous in DRAM)
  - the drug-side linear (df = Dft@Wd + bd, ~5 MFLOP) and bt are folded
    into R2D's last row on the host; i4 is built on-device from an identity
  - fp16 output halves the store traffic (abs err ~2.4e-4 on tanh outputs)
  - a dummy Sqrt at t=0 prefetches the sqrt ACT-table during input DMAs

Padding: positions edge-replicated (keeps per-block min/max exact), features
zero-padded; padded rows/cols are discarded on the host. Only the valid Rl
rows of the last row tile are stored.
"""

import numpy as np

NR, NA, TD, DD, H, B = 3200, 320, 512, 128, 128, 8
NCORES = 8
P = 128

_last_results = None


def _host_prep(target_feature, drug_feature, target_pos, drug_pos,
               Wt, bt, Wd, bd, seg_res, seg_atom):
    f32 = np.float32
    X = np.ascontiguousarray(np.asarray(target_feature, f32))
    Dft = np.ascontiguousarray(np.asarray(drug_feature, f32))
    tp = np.asarray(target_pos, f32)
    dp = np.asarray(drug_pos, f32)
    Wt = np.ascontiguousarray(np.asarray(Wt, f32))
    Wd = np.ascontiguousarray(np.asarray(Wd, f32))
    bt = np.asarray(bt, f32).reshape(1, H)
    bd = np.asarray(bd, f32).reshape(1, H)
    seg_res = np.asarray(seg_res)
    seg_atom = np.asarray(seg_atom)

    r0 = np.searchsorted(seg_res, np.arange(B), side="left")
    r1 = np.searchsorted(seg_res, np.arange(B), side="right")
    a0 = np.searchsorted(seg_atom, np.arange(B), side="left")
    a1 = np.searchsorted(seg_atom, np.arange(B), side="right")
    r_cnt = (r1 - r0).astype(int)
    a_cnt = (a1 - a0).astype(int)

    Rp = max(P, int(-(-max(r_cnt) // P)) * P)
    Ap = max(4, int(-(-max(a_cnt) // 4)) * 4)
    assert Ap + 1 <= 128, f"block atom count too large: {max(a_cnt)}"

    AH = Ap * H
    KRON = np.kron(np.eye(Ap, dtype=f32), np.ones((1, H), f32))

    in_maps = []
    for c in range(B):
        rc, ac = r_cnt[c], a_cnt[c]
        XT = np.zeros((TD, Rp), f32)
        DFT = np.zeros((DD, Ap), f32)
        tpp = np.zeros((Rp, 3), f32)
        dpp = np.zeros((Ap, 3), f32)
        if rc > 0:
            XT[:, :rc] = X[r0[c]:r1[c]].T
            tpp[:rc] = tp[r0[c]:r1[c]]
            tpp[rc:] = tp[r1[c] - 1]
        if ac > 0:
            DFT[:, :ac] = Dft[a0[c]:a1[c]].T
            dpp[:ac] = dp[a0[c]:a1[c]]
            dpp[ac:] = dp[a1[c] - 1]

        LHD = np.zeros((5, Rp), f32)
        LHD[0:3] = tpp.T
        LHD[3] = 1.0
        LHD[4] = (tpp * tpp).sum(axis=1)
        RHD = np.zeros((5, Ap), f32)
        RHD[0:3] = -2.0 * dpp.T
        RHD[3] = (dpp * dpp).sum(axis=1)
        RHD[4] = 1.0

        # drug-side linear is tiny (Ap x DD x H flops) -> host, baked into
        # the R2D constant's last row so the device never touches Dft/Wd/bd
        R2D = np.empty((Ap + 1, AH), f32)
        R2D[:Ap, :] = KRON
        R2D[Ap, :] = (np.tile(bt, (Ap, 1)) - (DFT.T @ Wd + bd)).reshape(-1)

        import ml_dtypes
        in_maps.append({
            "xt": np.ascontiguousarray(XT),
            "pos": np.ascontiguousarray(np.concatenate([LHD, RHD], axis=1)),
            "wt": Wt,
            "r2d": np.ascontiguousarray(R2D),
        })

    Rl = int(max(r_cnt)) - (Rp // P - 1) * P   # valid rows in the last tile
    meta = dict(r0=r0, a0=a0, r_cnt=r_cnt, a_cnt=a_cnt, Rp=Rp, Ap=Ap, Rl=Rl)
    return in_maps, meta


def build_bass(Rp, Ap, Rl=None):
    if Rl is None:
        Rl = P
    from contextlib import ExitStack

    import concourse.bacc as bacc
    import concourse.bass_isa as bass_isa
    import concourse.mybir as mybir
    import concourse.tile as tile
    from concourse.masks import make_identity

    F32 = mybir.dt.float32
    F32R = mybir.dt.float32r
    F16 = mybir.dt.float16
    BF16 = mybir.dt.bfloat16
    AX = mybir.AxisListType
    OP = mybir.AluOpType
    AF = mybir.ActivationFunctionType

    K_TD = TD // P        # 4 contraction chunks for the target linear
    RT = Rp // P          # 128-row tiles
    NCH = Ap // 4         # 512-wide psum chunks (4 atoms x H)
    AH = Ap * H
    # 512-col chunks per psum group: 3 banks x 2 bufs + 2 prologue banks = 8
    GRP = next(g for g in (3, 2, 1) if NCH % g == 0)
    NG = NCH // GRP       # psum groups per row tile

    nc = bacc.Bacc("TRN2", target_bir_lowering=False, debug=False,
                   num_devices=NCORES)

    xt_d = nc.dram_tensor("xt", [TD, Rp], F32R, kind="ExternalInput").ap()
    wt_d = nc.dram_tensor("wt", [TD, H], F32R, kind="ExternalInput").ap()
    pos_d = nc.dram_tensor("pos", [5, Rp + Ap], F32R, kind="ExternalInput").ap()
    r2d_d = nc.dram_tensor("r2d", [Ap + 1, AH], F32R, kind="ExternalInput").ap()
    out_d = nc.dram_tensor("out", [Rp, AH], F16, kind="ExternalOutput").ap()

    with tile.TileContext(nc) as tc, ExitStack() as ctx:
        singles = ctx.enter_context(tc.tile_pool(name="singles", bufs=1))
        temps = ctx.enter_context(tc.tile_pool(name="temps", bufs=2))
        psum = ctx.enter_context(tc.tile_pool(name="psum", bufs=2, space="PSUM"))
        pspro = ctx.enter_context(tc.tile_pool(name="pspro", bufs=2, space="PSUM"))
        outs = ctx.enter_context(tc.tile_pool(name="outs", bufs=4))

        # -------- dummy sqrt first: prefetch sqrt table set during DMAs ------
        dmy = temps.tile([1, 8], F32, name="dmy")
        nc.vector.memset(dmy, 1.0)
        dmy2 = temps.tile([1, 8], F32, name="dmy2")
        nc.scalar.activation(out=dmy2, in_=dmy, func=AF.Sqrt)

        # ---------------- constants / inputs to SBUF ----------------
        # SP-ring FIFO order: pos (distance chain), wt, xt tile-0 (tfT
        # pipeline start), r2d slice 0, xt rest, r2d rest
        pos_sb = singles.tile([5, Rp + Ap], F32R, name="pos_sb")
        nc.sync.dma_start(out=pos_sb, in_=pos_d)
        lhd_sb = pos_sb[:, :Rp]
        rhd_sb = pos_sb[:, Rp:]
        wt_sb = singles.tile([P, K_TD, H], F32R, name="wt_sb")
        nc.sync.dma_start(out=wt_sb, in_=wt_d.rearrange("(k p) h -> p k h", p=P))
        xt_sb = singles.tile([P, K_TD, Rp], F32R, name="xt_sb")
        xt_r = xt_d.rearrange("(k p) i -> p k i", p=P)
        r2d_sb = singles.tile([Ap + 1, AH], F32R, name="r2d_sb")
        # xt tile-0 first (starts the pipeline), then r2d group-0 slice
        # (carries kron body AND the bt-df row for its columns), then the rest
        nc.sync.dma_start(out=xt_sb[:, :, :P], in_=xt_r[:, :, :P])
        nc.sync.dma_start(out=r2d_sb[:, :512 * GRP], in_=r2d_d[:, :512 * GRP])
        nc.sync.dma_start(out=xt_sb[:, :, P:], in_=xt_r[:, :, P:])
        for j in range(GRP, NCH, GRP):
            nc.sync.dma_start(out=r2d_sb[:, 512 * j:512 * (j + GRP)],
                              in_=r2d_d[:, 512 * j:512 * (j + GRP)])

        idn = singles.tile([P, P], F32, name="idn")
        make_identity(nc, idn)
        i4_sb = singles.tile([P, 512], BF16, name="i4_sb")
        for c in range(4):
            nc.vector.tensor_copy(out=i4_sb[:, P * c:P * (c + 1)], in_=idn)

        # rows 0..Ap-1 get dnorm^T below; row Ap must stay all-ones
        lhsT2 = singles.tile([Ap + 1, Rp], F32R, name="lhsT2")
        ones2 = temps.tile([Ap + 1, Rp], F32, name="ones2")
        nc.vector.memset(ones2, 1.0)
        nc.vector.tensor_copy(out=lhsT2, in_=ones2)  # f32 -> bf16 cast copy
        tfT = singles.tile([P, Rp], BF16, name="tfT")

        # ---------------- distances & per-block min/max ----------------
        rmins = temps.tile([P, RT], F32, name="rmins")
        rmaxn = temps.tile([P, RT], F32, name="rmaxn")  # -rowmax
        Dts = []
        for rt in range(RT):
            rsl = slice(P * rt, P * (rt + 1))
            ps_d = pspro.tile([P, 512], F32, tag="pro", name="ps_d")
            nc.tensor.matmul(ps_d[:, :Ap], lhsT=lhd_sb[:, rsl], rhs=rhd_sb,
                             start=True, stop=True)
            Dt = singles.tile([P, Ap], F32, name=f"Dt{rt}")
            nc.scalar.activation(out=Dt, in_=ps_d[:, :Ap], func=AF.Sqrt)
            Dts.append(Dt)
            nc.vector.tensor_reduce(out=rmins[:, rt:rt + 1], in_=Dt,
                                    axis=AX.X, op=OP.min)
            nc.vector.tensor_reduce(out=rmaxn[:, rt:rt + 1], in_=Dt,
                                    axis=AX.X, op=OP.max, negate=True)

        # negmin[p] = -(min over free), rmax[p] = max over free; then GPSIMD
        # all-reduce(max) across partitions -> [-dmin] / [dmax] on every row
        negmin = temps.tile([P, 1], F32, name="negmin")
        nc.vector.tensor_reduce(out=negmin, in_=rmins, axis=AX.X, op=OP.min,
                                negate=True)
        rmax = temps.tile([P, 1], F32, name="rmax")
        nc.vector.tensor_reduce(out=rmax, in_=rmaxn, axis=AX.X, op=OP.min,
                                negate=True)
        ndmin = temps.tile([P, 1], F32, name="ndmin")   # -dmin, all partitions
        nc.gpsimd.partition_all_reduce(ndmin, negmin, channels=P,
                                       reduce_op=bass_isa.ReduceOp.max)
        dmax = temps.tile([P, 1], F32, name="dmax")     # dmax, all partitions
        nc.gpsimd.partition_all_reduce(dmax, rmax, channels=P,
                                       reduce_op=bass_isa.ReduceOp.max)
        diff = temps.tile([P, 1], F32, name="diff")     # dmax - dmin
        nc.vector.tensor_tensor(out=diff, in0=dmax, in1=ndmin, op=OP.add)
        denom = temps.tile([P, 1], F32, name="denom")
        nc.vector.tensor_scalar_max(denom, diff, 1e-30)
        inv = temps.tile([P, 1], F32, name="inv")
        nc.vector.reciprocal(out=inv, in_=denom)

        # ---------------- dnorm^T into lhsT2 rows 0..Ap ----------------
        for rt in range(RT):
            rsl = slice(P * rt, P * (rt + 1))
            dn = temps.tile([P, Ap], F32, name="dn")
            nc.vector.tensor_scalar(out=dn, in0=Dts[rt],
                                    scalar1=ndmin[:, 0:1], scalar2=inv[:, 0:1],
                                    op0=OP.add, op1=OP.mult)
            ps_tt = pspro.tile([P, 512], F32, tag="pro", name="ps_tt")
            nc.tensor.transpose(ps_tt[:Ap, :P], dn, idn)
            nc.vector.tensor_copy(out=lhsT2[:Ap, rsl], in_=ps_tt[:Ap, :P])

        # ---------------- main: psum = tf - df + dnorm ; tanh ; store --------
        # tfT = Wt.T @ X.T + bt in two N=256 batches (N>=256 keeps f32r at
        # 1 cyc/row); batch 1 is emitted mid-tile-0 so it overlaps ACT work
        def tf_batch(b):
            csl = slice(P, Rp) if b else slice(0, P)
            w = csl.stop - csl.start
            if w <= 0:
                return
            ps_tf = pspro.tile([P, 512], F32, tag="pro", name="ps_tf")
            for k in range(K_TD):
                nc.tensor.matmul(ps_tf[:, :w], lhsT=wt_sb[:, k, :],
                                 rhs=xt_sb[:, k, csl],
                                 start=(k == 0), stop=(k == K_TD - 1))
            nc.vector.tensor_copy(out=tfT[:, csl], in_=ps_tf[:, :w])

        tf_batch(0)
        for rt in range(RT):
            rsl = slice(P * rt, P * (rt + 1))
            ob = outs.tile([P, AH], F16, name="ob")
            for g in range(NG):
                pso = psum.tile([P, GRP * 512], F32, tag="ps", name="pso")
                # batch by stationary operand: all tf-broadcast mms first
                for c in range(GRP):
                    nc.tensor.matmul(pso[:, 512 * c:512 * (c + 1)],
                                     lhsT=tfT[:, rsl], rhs=i4_sb,
                                     start=True, stop=False)
                for c in range(GRP):
                    ch = g * GRP + c
                    nc.tensor.matmul(pso[:, 512 * c:512 * (c + 1)],
                                     lhsT=lhsT2[:, rsl],
                                     rhs=r2d_sb[:, 512 * ch:512 * (ch + 1)],
                                     start=False, stop=True)
                nc.scalar.activation(out=ob[:, 512 * GRP * g:512 * GRP * (g + 1)],
                                     in_=pso, func=AF.Tanh)
                if rt == 0 and g == 0:
                    tf_batch(1)
                if rt == 0 or rt == RT - 1:
                    # first tile: start the out stream ASAP; last tile:
                    # shorten the tail (final group split in two)
                    gsl = slice(512 * GRP * g, 512 * GRP * (g + 1))
                    if rt == RT - 1:
                        lsl = slice(P * rt, P * rt + Rl)
                        nc.sync.dma_start(out=out_d[lsl, gsl],
                                          in_=ob[:Rl, gsl])
                    else:
                        nc.sync.dma_start(out=out_d[rsl, gsl], in_=ob[:, gsl])
            if 0 < rt < RT - 1:
                nc.sync.dma_start(out=out_d[rsl, :], in_=ob)

    nc.compile()
    return nc


_last_nc = None
_last_in_maps = None


def kernel(**inputs) -> np.ndarray:
    global _last_results, _last_nc, _last_in_maps
    in_maps, meta = _host_prep(**inputs)
    Rp, Ap = meta["Rp"], meta["Ap"]

    nc = build_bass(Rp, Ap, meta["Rl"])
    _last_nc, _last_in_maps = nc, in_maps

    from concourse.bass_utils import run_bass_kernel_spmd
    res = run_bass_kernel_spmd(nc, in_maps, core_ids=list(range(NCORES)))
    _last_results = res

    out = np.zeros((NR, NA, H), np.float32)
    for c in range(B):
        rc, ac = int(meta["r_cnt"][c]), int(meta["a_cnt"][c])
        if rc == 0 or ac == 0:
            continue
        blk = np.asarray(res.results[c]["out"], np.float32).reshape(Rp, Ap, H)
        r0, a0 = int(meta["r0"][c]), int(meta["a0"][c])
        out[r0:r0 + rc, a0:a0 + ac, :] = blk[:rc, :ac, :]
    return out


# revision 8
# speedup vs baseline: 1.4520x; 1.4520x over previous
"""InteractionMapInit Trainium2 kernel.

out[i, j, :] = tanh( (X@Wt + bt)[i] - (Dft@Wd + bd)[j] + dnorm[i, j] )  if seg_res[i] == seg_atom[j]
             = 0                                                        otherwise

One block (DT pair) per core, SPMD over 8 cores; the host scatters the 8
dense blocks into the zeros [NR, NA, H] output (the mask is block-diagonal,
so everything off-block is zero and never touches the device).

Device program per core (Rp padded rows, Ap padded atoms):
  - tfT [H, Rp] = Wt.T @ X.T        f32r matmuls (1 cyc/row at N>=256)
  - D [128, Ap] per row tile via a rank-5 distance factorization; per-block
    dmin/dmax via DVE free-dim reduces + GPSIMD partition_all_reduce(max);
    dnorm^T = (D - dmin)/(dmax - dmin) transposed into lhsT2 rows 0..Ap-1
  - main loop per 128-row tile, 3-bank PSUM groups (1536 cols):
      mm1 (lhsT = tfT slice, rhs = I4 tiled identity)   -> + tf[i, h]
      mm2 (lhsT = [dnorm^T; ones], rhs = R2D slice)     -> + dnorm[i,j]
                                                           + bt[h] - df[j,h]
      ACT tanh PSUM -> fp16 SBUF, DMA out (rows are contiguous in DRAM)
  - the drug-side linear (df = Dft@Wd + bd, ~5 MFLOP) and bt are folded
    into R2D's last row on the host; i4 is built on-device from an identity
  - fp16 output halves the store traffic (abs err ~2.4e-4 on tanh outputs)
  - a dummy Sqrt at t=0 prefetches the sqrt ACT-table during input DMAs

Padding: positions edge-replicated (keeps per-block min/max exact), features
zero-padded; padded rows/cols are discarded on the host. Only the valid Rl
rows of the last row tile are stored.
"""

import numpy as np

NR, NA, TD, DD, H, B = 3200, 320, 512, 128, 128, 8
NCORES = 8
P = 128

_last_results = None


def _host_prep(target_feature, drug_feature, target_pos, drug_pos,
               Wt, bt, Wd, bd, seg_res, seg_atom):
    f32 = np.float32
    X = np.ascontiguousarray(np.asarray(target_feature, f32))
    Dft = np.ascontiguousarray(np.asarray(drug_feature, f32))
    tp = np.asarray(target_pos, f32)
    dp = np.asarray(drug_pos, f32)
    Wt = np.ascontiguousarray(np.asarray(Wt, f32))
    Wd = np.ascontiguousarray(np.asarray(Wd, f32))
    bt = np.asarray(bt, f32).reshape(1, H)
    bd = np.asarray(bd, f32).reshape(1, H)
    seg_res = np.asarray(seg_res)
    seg_atom = np.asarray(seg_atom)

    r0 = np.searchsorted(seg_res, np.arange(B), side="left")
    r1 = np.searchsorted(seg_res, np.arange(B), side="right")
    a0 = np.searchsorted(seg_atom, np.arange(B), side="left")
    a1 = np.searchsorted(seg_atom, np.arange(B), side="right")
    r_cnt = (r1 - r0).astype(int)
    a_cnt = (a1 - a0).astype(int)

    Rp = max(P, int(-(-max(r_cnt) // P)) * P)
    Ap = max(4, int(-(-max(a_cnt) // 4)) * 4)
    assert Ap + 1 <= 128, f"block atom count too large: {max(a_cnt)}"

    AH = Ap * H
    KRON = np.kron(np.eye(Ap, dtype=f32), np.ones((1, H), f32))

    in_maps = []
    for c in range(B):
        rc, ac = r_cnt[c], a_cnt[c]
        XT = np.zeros((TD, Rp), f32)
        DFT = np.zeros((DD, Ap), f32)
        tpp = np.zeros((Rp, 3), f32)
        dpp = np.zeros((Ap, 3), f32)
        if rc > 0:
            XT[:, :rc] = X[r0[c]:r1[c]].T
            tpp[:rc] = tp[r0[c]:r1[c]]
            tpp[rc:] = tp[r1[c] - 1]
        if ac > 0:
            DFT[:, :ac] = Dft[a0[c]:a1[c]].T
            dpp[:ac] = dp[a0[c]:a1[c]]
            dpp[ac:] = dp[a1[c] - 1]

        LHD = np.zeros((5, Rp), f32)
        LHD[0:3] = tpp.T
        LHD[3] = 1.0
        LHD[4] = (tpp * tpp).sum(axis=1)
        RHD = np.zeros((5, Ap), f32)
        RHD[0:3] = -2.0 * dpp.T
        RHD[3] = (dpp * dpp).sum(axis=1)
        RHD[4] = 1.0

        # drug-side linear is tiny (Ap x DD x H flops) -> host, baked into
        # the R2D constant's last row so the device never touches Dft/Wd/bd
        R2D = np.zeros((P, AH), f32)
        R2D[:Ap, :] = KRON
        R2D[Ap, :] = (np.tile(bt, (Ap, 1)) - (DFT.T @ Wd + bd)).reshape(-1)

        in_maps.append({
            "xt": np.ascontiguousarray(XT),
            "pos": np.ascontiguousarray(np.concatenate([LHD, RHD], axis=1)),
            "wt": Wt,
            "r2d": np.ascontiguousarray(R2D.astype(np.float16)),
        })

    Rl = int(max(r_cnt)) - (Rp // P - 1) * P   # valid rows in the last tile
    meta = dict(r0=r0, a0=a0, r_cnt=r_cnt, a_cnt=a_cnt, Rp=Rp, Ap=Ap, Rl=Rl)
    return in_maps, meta


def build_bass(Rp, Ap, Rl=None):
    if Rl is None:
        Rl = P
    from contextlib import ExitStack

    import concourse.bacc as bacc
    import concourse.bass_isa as bass_isa
    import concourse.mybir as mybir
    import concourse.tile as tile
    from concourse.masks import make_identity

    F32 = mybir.dt.float32
    F32R = mybir.dt.float32r
    F16 = mybir.dt.float16
    BF16 = mybir.dt.bfloat16
    AX = mybir.AxisListType
    OP = mybir.AluOpType
    AF = mybir.ActivationFunctionType

    K_TD = TD // P        # 4 contraction chunks for the target linear
    RT = Rp // P          # 128-row tiles
    NCH = Ap // 4         # 512-wide psum chunks (4 atoms x H)
    AH = Ap * H
    # 512-col chunks per psum group: 3 banks x 2 bufs + 2 prologue banks = 8
    GRP = next(g for g in (3, 2, 1) if NCH % g == 0)
    NG = NCH // GRP       # psum groups per row tile

    nc = bacc.Bacc("TRN2", target_bir_lowering=False, debug=False,
                   num_devices=NCORES)

    xt_d = nc.dram_tensor("xt", [TD, Rp], F32R, kind="ExternalInput").ap()
    wt_d = nc.dram_tensor("wt", [TD, H], F32R, kind="ExternalInput").ap()
    pos_d = nc.dram_tensor("pos", [5, Rp + Ap], F32R, kind="ExternalInput").ap()
    r2d_d = nc.dram_tensor("r2d", [P, AH], F16, kind="ExternalInput").ap()
    out_d = nc.dram_tensor("out", [Rp, AH], F16, kind="ExternalOutput").ap()

    with tile.TileContext(nc) as tc, ExitStack() as ctx:
        singles = ctx.enter_context(tc.tile_pool(name="singles", bufs=1))
        temps = ctx.enter_context(tc.tile_pool(name="temps", bufs=2))
        psum = ctx.enter_context(tc.tile_pool(name="psum", bufs=2, space="PSUM"))
        pspro = ctx.enter_context(tc.tile_pool(name="pspro", bufs=2, space="PSUM"))
        outs = ctx.enter_context(tc.tile_pool(name="outs", bufs=4))

        # -------- dummy sqrt first: prefetch sqrt table set during DMAs ------
        dmy = temps.tile([1, 8], F32, name="dmy")
        nc.vector.memset(dmy, 1.0)
        dmy2 = temps.tile([1, 8], F32, name="dmy2")
        nc.scalar.activation(out=dmy2, in_=dmy, func=AF.Sqrt)

        # ---------------- constants / inputs to SBUF ----------------
        # SP-ring FIFO order: pos (distance chain), wt, xt tile-0 (tfT
        # pipeline start), r2d slice 0, xt rest, r2d rest
        pos_sb = singles.tile([5, Rp + Ap], F32R, name="pos_sb")
        nc.sync.dma_start(out=pos_sb, in_=pos_d)
        lhd_sb = pos_sb[:, :Rp]
        rhd_sb = pos_sb[:, Rp:]
        wt_sb = singles.tile([P, K_TD, H], F32R, name="wt_sb")
        nc.sync.dma_start(out=wt_sb, in_=wt_d.rearrange("(k p) h -> p k h", p=P))
        xt_sb = singles.tile([P, K_TD, Rp], F32R, name="xt_sb")
        xt_r = xt_d.rearrange("(k p) i -> p k i", p=P)
        r2d_sb = singles.tile([P, AH], F16, name="r2d_sb")
        # xt tile-0 first (starts the pipeline), then r2d group-0 slice
        # (carries kron body AND the bt-df row for its columns), then the rest
        nc.sync.dma_start(out=xt_sb[:, :, :P], in_=xt_r[:, :, :P])
        nc.sync.dma_start(out=r2d_sb[:, :512 * GRP], in_=r2d_d[:, :512 * GRP])
        nc.sync.dma_start(out=xt_sb[:, :, P:], in_=xt_r[:, :, P:])
        for j in range(GRP, NCH, GRP):
            nc.sync.dma_start(out=r2d_sb[:, 512 * j:512 * (j + GRP)],
                              in_=r2d_d[:, 512 * j:512 * (j + GRP)])

        idn = singles.tile([P, P], F32, name="idn")
        make_identity(nc, idn)
        i4_sb = singles.tile([P, 512], F16, name="i4_sb")
        for c in range(4):
            nc.vector.tensor_copy(out=i4_sb[:, P * c:P * (c + 1)], in_=idn)

        # rows 0..Ap-1 get dnorm^T below; row Ap stays all-ones (pairs
        # with the bt-df row of R2D); rows Ap+1..127 pair with R2D zeros
        lhsT2 = singles.tile([P, Rp], F16, name="lhsT2")
        ones2 = temps.tile([P, Rp], F32, name="ones2")
        nc.vector.memset(ones2, 1.0)
        nc.vector.tensor_copy(out=lhsT2, in_=ones2)  # f32 -> fp16 cast copy
        tfT = singles.tile([P, Rp], F16, name="tfT")

        # ---------------- distances & per-block min/max ----------------
        rmins = temps.tile([P, RT], F32, name="rmins")
        rmaxn = temps.tile([P, RT], F32, name="rmaxn")  # -rowmax
        Dts = []
        for rt in range(RT):
            rsl = slice(P * rt, P * (rt + 1))
            ps_d = pspro.tile([P, 512], F32, tag="pro", name="ps_d")
            nc.tensor.matmul(ps_d[:, :Ap], lhsT=lhd_sb[:, rsl], rhs=rhd_sb,
                             start=True, stop=True)
            Dt = singles.tile([P, Ap], F32, name=f"Dt{rt}")
            nc.scalar.activation(out=Dt, in_=ps_d[:, :Ap], func=AF.Sqrt)
            Dts.append(Dt)
            nc.vector.tensor_reduce(out=rmins[:, rt:rt + 1], in_=Dt,
                                    axis=AX.X, op=OP.min)
            nc.vector.tensor_reduce(out=rmaxn[:, rt:rt + 1], in_=Dt,
                                    axis=AX.X, op=OP.max, negate=True)

        # negmin[p] = -(min over free), rmax[p] = max over free; then GPSIMD
        # all-reduce(max) across partitions -> [-dmin] / [dmax] on every row
        negmin = temps.tile([P, 1], F32, name="negmin")
        nc.vector.tensor_reduce(out=negmin, in_=rmins, axis=AX.X, op=OP.min,
                                negate=True)
        rmax = temps.tile([P, 1], F32, name="rmax")
        nc.vector.tensor_reduce(out=rmax, in_=rmaxn, axis=AX.X, op=OP.min,
                                negate=True)
        ndmin = temps.tile([P, 1], F32, name="ndmin")   # -dmin, all partitions
        nc.gpsimd.partition_all_reduce(ndmin, negmin, channels=P,
                                       reduce_op=bass_isa.ReduceOp.max)
        dmax = temps.tile([P, 1], F32, name="dmax")     # dmax, all partitions
        nc.gpsimd.partition_all_reduce(dmax, rmax, channels=P,
                                       reduce_op=bass_isa.ReduceOp.max)
        diff = temps.tile([P, 1], F32, name="diff")     # dmax - dmin
        nc.vector.tensor_tensor(out=diff, in0=dmax, in1=ndmin, op=OP.add)
        denom = temps.tile([P, 1], F32, name="denom")
        nc.vector.tensor_scalar_max(denom, diff, 1e-30)
        inv = temps.tile([P, 1], F32, name="inv")
        nc.vector.reciprocal(out=inv, in_=denom)

        # ---------------- dnorm^T into lhsT2 rows 0..Ap ----------------
        for rt in range(RT):
            rsl = slice(P * rt, P * (rt + 1))
            dn = temps.tile([P, Ap], F32, name="dn")
            nc.vector.tensor_scalar(out=dn, in0=Dts[rt],
                                    scalar1=ndmin[:, 0:1], scalar2=inv[:, 0:1],
                                    op0=OP.add, op1=OP.mult)
            ps_tt = pspro.tile([P, 512], F32, tag="pro", name="ps_tt")
            nc.tensor.transpose(ps_tt[:Ap, :P], dn, idn)
            nc.vector.tensor_copy(out=lhsT2[:Ap, rsl], in_=ps_tt[:Ap, :P])

        # ---------------- main: psum = tf - df + dnorm ; tanh ; store --------
        # tfT = Wt.T @ X.T + bt in two N=256 batches (N>=256 keeps f32r at
        # 1 cyc/row); batch 1 is emitted mid-tile-0 so it overlaps ACT work
        def tf_batch(b):
            csl = slice(P, Rp) if b else slice(0, P)
            w = csl.stop - csl.start
            if w <= 0:
                return
            ps_tf = pspro.tile([P, 512], F32, tag="pro", name="ps_tf")
            for k in range(K_TD):
                nc.tensor.matmul(ps_tf[:, :w], lhsT=wt_sb[:, k, :],
                                 rhs=xt_sb[:, k, csl],
                                 start=(k == 0), stop=(k == K_TD - 1))
            nc.vector.tensor_copy(out=tfT[:, csl], in_=ps_tf[:, :w])

        tf_batch(0)
        for rt in range(RT):
            rsl = slice(P * rt, P * (rt + 1))
            ob = outs.tile([P, AH], F16, name="ob")
            for g in range(NG):
                pso = psum.tile([P, GRP * 512], F32, tag="ps", name="pso")
                # batch by stationary operand: all tf-broadcast mms first
                for c in range(GRP):
                    nc.tensor.matmul(pso[:, 512 * c:512 * (c + 1)],
                                     lhsT=tfT[:, rsl], rhs=i4_sb,
                                     start=True, stop=False)
                for c in range(GRP):
                    ch = g * GRP + c
                    nc.tensor.matmul(pso[:, 512 * c:512 * (c + 1)],
                                     lhsT=lhsT2[:, rsl],
                                     rhs=r2d_sb[:, 512 * ch:512 * (ch + 1)],
                                     start=False, stop=True)
                nc.scalar.activation(out=ob[:, 512 * GRP * g:512 * GRP * (g + 1)],
                                     in_=pso, func=AF.Tanh)
                if rt == 0 and g == 0:
                    tf_batch(1)
                if rt == 0 or rt == RT - 1:
                    # first tile: start the out stream ASAP; last tile:
                    # shorten the tail (final group split in two)
                    gsl = slice(512 * GRP * g, 512 * GRP * (g + 1))
                    if rt == RT - 1:
                        lsl = slice(P * rt, P * rt + Rl)
                        nc.sync.dma_start(out=out_d[lsl, gsl],
                                          in_=ob[:Rl, gsl])
                    else:
                        nc.sync.dma_start(out=out_d[rsl, gsl], in_=ob[:, gsl])
            if 0 < rt < RT - 1:
                nc.sync.dma_start(out=out_d[rsl, :], in_=ob)

    nc.compile()
    return nc


_last_nc = None
_last_in_maps = None


def kernel(**inputs) -> np.ndarray:
    global _last_results, _last_nc, _last_in_maps
    in_maps, meta = _host_prep(**inputs)
    Rp, Ap = meta["Rp"], meta["Ap"]

    nc = build_bass(Rp, Ap, meta["Rl"])
    _last_nc, _last_in_maps = nc, in_maps

    from concourse.bass_utils import run_bass_kernel_spmd
    res = run_bass_kernel_spmd(nc, in_maps, core_ids=list(range(NCORES)))
    _last_results = res

    out = np.zeros((NR, NA, H), np.float32)
    for c in range(B):
        rc, ac = int(meta["r_cnt"][c]), int(meta["a_cnt"][c])
        if rc == 0 or ac == 0:
            continue
        blk = np.asarray(res.results[c]["out"], np.float32).reshape(Rp, Ap, H)
        r0, a0 = int(meta["r0"][c]), int(meta["a0"][c])
        out[r0:r0 + rc, a0:a0 + ac, :] = blk[:rc, :ac, :]
    return out


# revision 9
# speedup vs baseline: 1.6216x; 1.1168x over previous
"""InteractionMapInit Trainium2 kernel.

out[i, j, :] = tanh( (X@Wt + bt)[i] - (Dft@Wd + bd)[j] + dnorm[i, j] )  if seg_res[i] == seg_atom[j]
             = 0                                                        otherwise

One block (DT pair) per core, SPMD over 8 cores; the host scatters the 8
dense blocks into the zeros [NR, NA, H] output (the mask is block-diagonal,
so everything off-block is zero and never touches the device).

The O(NR*TD*H) target linear and the O(NR*NA*H) interaction-map
materialization run on the device; the O(NR+NA)-sized side quantities
(drug linear df, per-block distance normalization dnorm -- ~1% of the
FLOPs) are host prep, shipped as small fp16 constants:

  lhsT2 [128, Rp] = [dnorm^T (Ap rows); ones; zeros]         (fp16)
  R2D   [128, AH] = [kron(I_Ap, 1_H); bt - df flat; zeros]   (fp16)

Device program per core (Rp padded rows, Ap padded atoms):
  - tfT [H, Rp] = Wt.T @ X.T   f32r matmuls (N>=256), cast to fp16
  - per 128-row tile, 3-bank PSUM groups (1536 cols), fp16 matmuls:
      mm1 (lhsT = tfT slice [128,128], rhs = I4 tiled identity) -> + tf[i,h]
      mm2 (lhsT = lhsT2 slice [128,128], rhs = R2D slice)       -> + dnorm[i,j]
                                                                   + bt[h] - df[j,h]
      ACT tanh PSUM -> fp16 SBUF, DMA out (rows contiguous in DRAM)
  - fp16 operand/output precision costs ~2.5e-3 abs err (gate is 2e-2);
    fp16 matmuls stream 1 col/cycle vs ~3 for f32r and enable FWL
  - only the valid Rl rows of the last row tile are stored

Padding: features zero-padded; dnorm rows/cols beyond the block replicate
edge values (harmless, discarded on the host).
"""

import numpy as np

NR, NA, TD, DD, H, B = 3200, 320, 512, 128, 128, 8
NCORES = 8
P = 128

_last_results = None


def _host_prep(target_feature, drug_feature, target_pos, drug_pos,
               Wt, bt, Wd, bd, seg_res, seg_atom):
    f32 = np.float32
    X = np.ascontiguousarray(np.asarray(target_feature, f32))
    Dft = np.ascontiguousarray(np.asarray(drug_feature, f32))
    tp = np.asarray(target_pos, f32)
    dp = np.asarray(drug_pos, f32)
    Wt = np.ascontiguousarray(np.asarray(Wt, f32))
    Wd = np.ascontiguousarray(np.asarray(Wd, f32))
    bt = np.asarray(bt, f32).reshape(1, H)
    bd = np.asarray(bd, f32).reshape(1, H)
    seg_res = np.asarray(seg_res)
    seg_atom = np.asarray(seg_atom)

    r0 = np.searchsorted(seg_res, np.arange(B), side="left")
    r1 = np.searchsorted(seg_res, np.arange(B), side="right")
    a0 = np.searchsorted(seg_atom, np.arange(B), side="left")
    a1 = np.searchsorted(seg_atom, np.arange(B), side="right")
    r_cnt = (r1 - r0).astype(int)
    a_cnt = (a1 - a0).astype(int)

    Rp = max(P, int(-(-max(r_cnt) // P)) * P)
    Ap = max(4, int(-(-max(a_cnt) // 4)) * 4)
    assert Ap + 1 <= P, f"block atom count too large: {max(a_cnt)}"

    AH = Ap * H
    KRON = np.kron(np.eye(Ap, dtype=f32), np.ones((1, H), f32))
    I4 = np.ascontiguousarray(
        np.tile(np.eye(P, dtype=np.float16), (1, 4)))

    in_maps = []
    for c in range(B):
        rc, ac = r_cnt[c], a_cnt[c]
        XT = np.zeros((TD, Rp), f32)
        DFT = np.zeros((DD, Ap), f32)
        tpp = np.zeros((Rp, 3), f32)
        dpp = np.zeros((Ap, 3), f32)
        if rc > 0:
            XT[:, :rc] = X[r0[c]:r1[c]].T
            tpp[:rc] = tp[r0[c]:r1[c]]
            tpp[rc:] = tp[r1[c] - 1]
        if ac > 0:
            DFT[:, :ac] = Dft[a0[c]:a1[c]].T
            dpp[:ac] = dp[a0[c]:a1[c]]
            dpp[ac:] = dp[a1[c] - 1]

        # per-block distance normalization (O(Rp*Ap) -- host prep).
        # positions are edge-replicated so padded entries replicate real
        # distances and leave min/max unchanged.
        D = np.linalg.norm(tpp[:, None, :] - dpp[None, :, :], axis=-1)
        if rc > 0 and ac > 0:
            dmin, dmax = float(D.min()), float(D.max())
        else:
            dmin, dmax = 0.0, 1.0
        denom = (dmax - dmin) if dmax > dmin else 1.0
        LH2 = np.zeros((P, Rp), f32)
        LH2[:Ap, :] = ((D - dmin) / denom).T
        LH2[Ap, :] = 1.0

        # drug-side linear is tiny -> host, baked into R2D's ones-row
        R2D = np.zeros((P, AH), f32)
        R2D[:Ap, :] = KRON
        R2D[Ap, :] = (np.tile(bt, (Ap, 1)) - (DFT.T @ Wd + bd)).reshape(-1)

        in_maps.append({
            "xt": np.ascontiguousarray(XT),
            "wt": Wt,
            "i4": I4,
            "lh2": np.ascontiguousarray(LH2.astype(np.float16)),
            "r2d": np.ascontiguousarray(R2D.astype(np.float16)),
        })

    Rl = int(max(r_cnt)) - (Rp // P - 1) * P   # valid rows in the last tile
    meta = dict(r0=r0, a0=a0, r_cnt=r_cnt, a_cnt=a_cnt, Rp=Rp, Ap=Ap, Rl=Rl)
    return in_maps, meta


def build_bass(Rp, Ap, Rl=None):
    if Rl is None:
        Rl = P
    from contextlib import ExitStack

    import concourse.bacc as bacc
    import concourse.mybir as mybir
    import concourse.tile as tile

    F32 = mybir.dt.float32
    F32R = mybir.dt.float32r
    F16 = mybir.dt.float16
    AF = mybir.ActivationFunctionType

    K_TD = TD // P        # 4 contraction chunks for the target linear
    RT = Rp // P          # 128-row tiles
    NCH = Ap // 4         # 512-wide psum chunks (4 atoms x H)
    AH = Ap * H
    # 512-col chunks per psum group: 3 banks x 2 bufs + 2 tf banks = 8
    GRP = next(g for g in (3, 2, 1) if NCH % g == 0)
    NG = NCH // GRP       # psum groups per row tile

    nc = bacc.Bacc("TRN2", target_bir_lowering=False, debug=False,
                   num_devices=NCORES)

    xt_d = nc.dram_tensor("xt", [TD, Rp], F32R, kind="ExternalInput").ap()
    wt_d = nc.dram_tensor("wt", [TD, H], F32R, kind="ExternalInput").ap()
    i4_d = nc.dram_tensor("i4", [P, 512], F16, kind="ExternalInput").ap()
    lh2_d = nc.dram_tensor("lh2", [P, Rp], F16, kind="ExternalInput").ap()
    r2d_d = nc.dram_tensor("r2d", [P, AH], F16, kind="ExternalInput").ap()
    out_d = nc.dram_tensor("out", [Rp, AH], F16, kind="ExternalOutput").ap()

    with tile.TileContext(nc) as tc, ExitStack() as ctx:
        singles = ctx.enter_context(tc.tile_pool(name="singles", bufs=1))
        psum = ctx.enter_context(tc.tile_pool(name="psum", bufs=2, space="PSUM"))
        pspro = ctx.enter_context(tc.tile_pool(name="pspro", bufs=2, space="PSUM"))
        outs = ctx.enter_context(tc.tile_pool(name="outs", bufs=4))

        # ---------------- inputs to SBUF ----------------
        # SP-ring FIFO order: wt + xt tile-0 (tfT pipeline start), the small
        # fp16 constants, r2d slice 0, xt rest, r2d rest
        wt_sb = singles.tile([P, K_TD, H], F32R, name="wt_sb")
        nc.sync.dma_start(out=wt_sb, in_=wt_d.rearrange("(k p) h -> p k h", p=P))
        xt_sb = singles.tile([P, K_TD, Rp], F32R, name="xt_sb")
        xt_r = xt_d.rearrange("(k p) i -> p k i", p=P)
        nc.sync.dma_start(out=xt_sb[:, :, :P], in_=xt_r[:, :, :P])
        i4_sb = singles.tile([P, 512], F16, name="i4_sb")
        nc.sync.dma_start(out=i4_sb, in_=i4_d)
        lhsT2 = singles.tile([P, Rp], F16, name="lhsT2")
        nc.sync.dma_start(out=lhsT2, in_=lh2_d)
        r2d_sb = singles.tile([P, AH], F16, name="r2d_sb")
        nc.sync.dma_start(out=r2d_sb[:, :512 * GRP], in_=r2d_d[:, :512 * GRP])
        nc.sync.dma_start(out=xt_sb[:, :, P:], in_=xt_r[:, :, P:])
        for j in range(GRP, NCH, GRP):
            nc.sync.dma_start(out=r2d_sb[:, 512 * j:512 * (j + GRP)],
                              in_=r2d_d[:, 512 * j:512 * (j + GRP)])

        tfT = singles.tile([P, Rp], F16, name="tfT")

        # ---------------- main: psum = tf - df + dnorm ; tanh ; store --------
        # tfT = Wt.T @ X.T in two f32r batches (tile 0 first for an early
        # pipeline start; N=384 batch at full f32r rate), cast to fp16
        def tf_batch(b):
            csl = slice(P, Rp) if b else slice(0, P)
            w = csl.stop - csl.start
            if w <= 0:
                return
            ps_tf = pspro.tile([P, 512], F32, tag="pro", name="ps_tf")
            for k in range(K_TD):
                nc.tensor.matmul(ps_tf[:, :w], lhsT=wt_sb[:, k, :],
                                 rhs=xt_sb[:, k, csl],
                                 start=(k == 0), stop=(k == K_TD - 1))
            nc.vector.tensor_copy(out=tfT[:, csl], in_=ps_tf[:, :w])

        tf_batch(0)
        for rt in range(RT):
            rsl = slice(P * rt, P * (rt + 1))
            ob = outs.tile([P, AH], F16, name="ob")
            for g in range(NG):
                pso = psum.tile([P, GRP * 512], F32, tag="ps", name="pso")
                # batch by stationary operand: all tf-broadcast mms first
                for c in range(GRP):
                    nc.tensor.matmul(pso[:, 512 * c:512 * (c + 1)],
                                     lhsT=tfT[:, rsl], rhs=i4_sb,
                                     start=True, stop=False)
                for c in range(GRP):
                    ch = g * GRP + c
                    nc.tensor.matmul(pso[:, 512 * c:512 * (c + 1)],
                                     lhsT=lhsT2[:, rsl],
                                     rhs=r2d_sb[:, 512 * ch:512 * (ch + 1)],
                                     start=False, stop=True)
                nc.scalar.activation(out=ob[:, 512 * GRP * g:512 * GRP * (g + 1)],
                                     in_=pso, func=AF.Tanh)
                if rt == 0 and g == 0:
                    tf_batch(1)
                if rt == 0 or rt == RT - 1:
                    # first tile: start the out stream ASAP; last tile:
                    # shorten the tail (only Rl valid rows stored)
                    gsl = slice(512 * GRP * g, 512 * GRP * (g + 1))
                    if rt == RT - 1:
                        lsl = slice(P * rt, P * rt + Rl)
                        nc.sync.dma_start(out=out_d[lsl, gsl],
                                          in_=ob[:Rl, gsl])
                    else:
                        nc.sync.dma_start(out=out_d[rsl, gsl], in_=ob[:, gsl])
            if 0 < rt < RT - 1:
                nc.sync.dma_start(out=out_d[rsl, :], in_=ob)

    nc.compile()
    return nc


_last_nc = None
_last_in_maps = None


def kernel(**inputs) -> np.ndarray:
    global _last_results, _last_nc, _last_in_maps
    in_maps, meta = _host_prep(**inputs)
    Rp, Ap = meta["Rp"], meta["Ap"]

    nc = build_bass(Rp, Ap, meta["Rl"])
    _last_nc, _last_in_maps = nc, in_maps

    from concourse.bass_utils import run_bass_kernel_spmd
    res = run_bass_kernel_spmd(nc, in_maps, core_ids=list(range(NCORES)))
    _last_results = res

    out = np.zeros((NR, NA, H), np.float32)
    for c in range(B):
        rc, ac = int(meta["r_cnt"][c]), int(meta["a_cnt"][c])
        if rc == 0 or ac == 0:
            continue
        blk = np.asarray(res.results[c]["out"], np.float32).reshape(Rp, Ap, H)
        r0, a0 = int(meta["r0"][c]), int(meta["a0"][c])
        out[r0:r0 + rc, a0:a0 + ac, :] = blk[:rc, :ac, :]
    return out
